# revision 1
# baseline (speedup 1.0000x reference)
"""TRN2 Bass kernel for causal multi-head attention with RoPE.

Problem: B=2, S=2048, HID=2048, NH=16, HD=128 (fp32).
Sharding: 8 cores = 2 (batch) x 4 (head-groups of 4 heads).
Each core computes q/k/v projections for its 4 heads (column-parallel),
RoPE, causal attention, and a row-parallel partial o_proj; the host sums
the 4 partials per batch.

Per-core device program (matmuls fp32r at full PE rate; P@V in bf16):
  Phase P: V = x @ Wv_g (natural layout, bf16 + ones column),
           QT/KT = (W x)^T with RoPE fused into the PSUM eviction
           (ACT copies PSUM->SBUF, DVE does full-width rotate ops),
           QT/KT spilled to DRAM (SBUF pressure).
  Phase A: chunk-outer loop; per (chunk, head): scores^T = KT^T Q with
           causal tile skipping and N-trimmed diagonal tiles, exp on ACT
           (scale fused), bf16 0/1 causal mask multiplied on GPSIMD,
           P@[V|1] accumulated in PSUM (ones column = softmax sums),
           per-partition normalize, PE transpose into attn_outT.
           o_proj for the finished seq-tiles is interleaved per chunk to
           fill PE dependency stalls.
"""
import os
import sys

if "/opt/trn_rl_repo" not in sys.path:
    sys.path.insert(0, "/opt/trn_rl_repo")

import numpy as np
import ml_dtypes

import concourse.bass as bass
import concourse.mybir as mybir
import concourse.tile as tile
from concourse import bacc
from concourse.bass_utils import run_bass_kernel_spmd
from concourse.masks import make_identity
from contextlib import ExitStack

P = 128
B, S, HID, NH = 2, 2048, 2048, 16
HD = HID // NH              # 128
H = 4                       # heads per core
DPC = H * HD                # 512 dims per core
KO = HID // P               # 16 contraction chunks
SC = S // 512               # 4 seq chunks of 512
ST = S // P                 # 16 seq tiles of 128
SCALE = 1.0 / float(np.sqrt(HD))

f32 = mybir.dt.float32
f32r = mybir.dt.float32r
bf16 = mybir.dt.bfloat16

_CACHED_NC = None


def build_nc():
    AF = mybir.ActivationFunctionType
    nc = bacc.Bacc(None, target_bir_lowering=False)

    xt = nc.declare_dram_parameter("xt", [P, KO, S], f32r, isOutput=False)
    wq = nc.declare_dram_parameter("wq", [H, P, KO, HD], f32r, isOutput=False)
    wk = nc.declare_dram_parameter("wk", [H, P, KO, HD], f32r, isOutput=False)
    wv = nc.declare_dram_parameter("wv", [P, KO, DPC], f32r, isOutput=False)
    wo = nc.declare_dram_parameter("wo", [P, H, HID], f32r, isOutput=False)
    cosf = nc.declare_dram_parameter("cosf", [P, S], f32, isOutput=False)
    sinf = nc.declare_dram_parameter("sinf", [P, S], f32, isOutput=False)
    bmask = nc.declare_dram_parameter("bmask", [P, H, 512], bf16, isOutput=False)
    out_p = nc.declare_dram_parameter("out_p", [S, HID], f32, isOutput=True)

    out3 = out_p.rearrange("(st p) n -> p st n", p=P)

    with tile.TileContext(nc) as tc:
        with ExitStack() as top:
            vpool = top.enter_context(tc.tile_pool(name="vpool", bufs=1))
            const = top.enter_context(tc.tile_pool(name="const", bufs=1))
            dram = top.enter_context(tc.tile_pool(name="dram", bufs=1, space="DRAM"))

            vsb = vpool.tile([P, ST, H, 132], bf16)
            nc.vector.memset(vsb[:, :, :, 128:132], 1.0)

            qt_sp = dram.tile([H, P, S], f32r)
            kt_sp = dram.tile([H, P, S], f32r)

            # ---------------- Phase P: projections ----------------
            with ExitStack() as ctx:
                xpool = ctx.enter_context(tc.tile_pool(name="xp", bufs=1))
                pp = ctx.enter_context(tc.tile_pool(name="pp", bufs=4, space="PSUM"))

                # x load interleaved seq-chunk-major so V matmuls can start
                # after the first chunk; alternate the two HWDGE queues.
                # wv half 0 is issued before x so the first V matmuls are not
                # starved behind the 16MB x stream.
                xs = xpool.tile([P, KO, S], f32r)
                # q/k weight pool opens early (bufs=1, 8KB) so the first
                # wq tile prefetches during the V phase
                wpool = ctx.enter_context(tc.tile_pool(name="wqk", bufs=1))
                with tc.tile_pool(name="wvp", bufs=2) as wvp:
                    wvts = [
                        wvp.tile([P, KO, DPC // 2], f32r, tag="wv", name=f"wv{vh}")
                        for vh in range(2)
                    ]
                    nc.sync.dma_start(wvts[0][:, :, 0:128], wv[:, :, 0:128])
                    nc.scalar.dma_start(wvts[0][:, :, 128:256], wv[:, :, 128:256])
                    for sc in range(SC):
                        for ko in range(KO):
                            eng = nc.sync if (ko % 2 == 0) else nc.scalar
                            eng.dma_start(xs[:, ko, sc * 512:(sc + 1) * 512],
                                          xt[:, ko, sc * 512:(sc + 1) * 512])
                        if sc == 0:
                            nc.sync.dma_start(wvts[1][:, :, 0:128], wv[:, :, 256:384])
                            nc.scalar.dma_start(wvts[1][:, :, 128:256], wv[:, :, 384:512])

                    for vh in range(2):
                        wvt = wvts[vh]
                        for st in range(ST):
                            ps = pp.tile([P, 256], f32, tag="vproj")
                            for ko in range(KO):
                                nc.tensor.matmul(
                                    ps[:],
                                    xs[:, ko, st * P:(st + 1) * P],
                                    wvt[:, ko],
                                    start=(ko == 0),
                                    stop=(ko == KO - 1),
                                )
                            nc.vector.tensor_copy(
                                vsb[:, st, vh * 2:(vh + 1) * 2, 0:128],
                                ps.rearrange("p (h d) -> p h d", h=2),
                            )

                cspool = ctx.enter_context(tc.tile_pool(name="cs", bufs=1))
                rtmp = ctx.enter_context(tc.tile_pool(name="rt", bufs=2))
                spill = ctx.enter_context(tc.tile_pool(name="sp", bufs=2))
                # full-height tables: cos duplicated halves; sin signed
                # (-sin rows 0:64, +sin rows 64:128) so the combine is one add
                cosT = cspool.tile([P, S], f32)
                sinT = cspool.tile([P, S], f32)
                nc.scalar.dma_start(cosT[:], cosf[:])
                nc.scalar.dma_start(sinT[:], sinf[:])

                if True:
                    for w4, sp_dram in ((wq, qt_sp), (wk, kt_sp)):
                        for h in range(H):
                            wt = wpool.tile([P, KO, HD], f32r, tag="w")
                            nc.sync.dma_start(wt[:], w4[h])
                            for sc in range(SC):
                                ssl = slice(sc * 512, (sc + 1) * 512)
                                ps = pp.tile([P, 512], f32, tag="proj")
                                for ko in range(KO):
                                    nc.tensor.matmul(
                                        ps[:],
                                        wt[:, ko],
                                        xs[:, ko, ssl],
                                        start=(ko == 0),
                                        stop=(ko == KO - 1),
                                    )
                                # RoPE eviction: partition-shifted reads are
                                # legal only with a PSUM operand, so the two
                                # rotate half-ops read ps directly; the combine
                                # runs full-width on SBUF.
                                t0 = rtmp.tile([P, 512], f32, tag="t0")
                                spt = spill.tile([P, 512], f32r, tag="spl")
                                nc.vector.tensor_mul(t0[0:64], ps[64:128], sinT[0:64, ssl])
                                nc.vector.tensor_mul(t0[64:128], ps[0:64], sinT[64:128, ssl])
                                nc.vector.tensor_mul(spt[:], ps[:], cosT[:, ssl])
                                nc.vector.tensor_add(spt[:], spt[:], t0[:])
                                nc.gpsimd.dma_start(sp_dram[h][:, ssl], spt[:])

            # ------------- Phase A: attention + interleaved o_proj -------------
            with ExitStack() as ctx:
                kpool = ctx.enter_context(tc.tile_pool(name="kp", bufs=1))
                qcpool = ctx.enter_context(tc.tile_pool(name="qc", bufs=2))
                ppool = ctx.enter_context(tc.tile_pool(name="ppool", bufs=6))
                stage = ctx.enter_context(tc.tile_pool(name="stage", bufs=4))
                aopool = ctx.enter_context(tc.tile_pool(name="ao", bufs=1))
                wopool = ctx.enter_context(tc.tile_pool(name="wop", bufs=1))
                ost = ctx.enter_context(tc.tile_pool(name="ost", bufs=4))
                spsum = ctx.enter_context(tc.tile_pool(name="sps", bufs=2, space="PSUM"))
                opsum = ctx.enter_context(tc.tile_pool(name="ops", bufs=2, space="PSUM"))
                opo = ctx.enter_context(tc.tile_pool(name="opo", bufs=2, space="PSUM"))

                bmt = const.tile([P, H, 512], bf16)
                nc.scalar.dma_start(bmt[:], bmask[:])
                zb = const.tile([P, 1], f32)
                nc.vector.memset(zb[:], 0.0)
                ones_col = const.tile([P, 1], bf16)
                nc.vector.memset(ones_col[:], 1.0)

                kall = kpool.tile([P, H, S], f32r)

                aot_c = [
                    aopool.tile([P, H, 512], f32r, tag=f"aot{c}", name=f"aot{c}")
                    for c in range(SC)
                ]

                def emit_oproj(cc):
                    for st4 in range(4):
                        st = cc * 4 + st4
                        for nch in range(4):
                            pso = opo.tile([P, 512], f32, tag="po", name="pso")
                            for dc in range(H):
                                nc.tensor.matmul(
                                    pso[:],
                                    aot_c[cc][:, dc, st4 * P:(st4 + 1) * P],
                                    wot[:, dc, nch * 512:(nch + 1) * 512],
                                    start=(dc == 0),
                                    stop=(dc == H - 1),
                                )
                            ob = ost.tile([P, 512], f32, tag="ob", name="ob")
                            nc.vector.tensor_copy(ob[:], pso[:])
                            nc.sync.dma_start(
                                out3[:, st, nch * 512:(nch + 1) * 512], ob[:]
                            )

                qcs = []
                for c in range(SC):
                    qc = qcpool.tile([P, H, 512], f32r, tag="qc", name=f"qc{c}")
                    qcs.append(qc)
                # reloads are sliced per 512-chunk and ordered chunk-major so
                # the first scores are gated on ~512KB, not the full 6MB
                for h in range(H):
                    eng = nc.scalar if h % 2 == 0 else nc.sync
                    eng.dma_start(qcs[0][:, h], qt_sp[h][:, 0:512])
                for cc in range(SC):
                    for h in range(H):
                        eng = nc.scalar if (cc * H + h) % 2 == 0 else nc.sync
                        eng.dma_start(kall[:, h, cc * 512:(cc + 1) * 512],
                                      kt_sp[h][:, cc * 512:(cc + 1) * 512])
                    if cc == 1:
                        wot = wopool.tile([P, H, HID], f32r)
                        nc.sync.dma_start(wot[:], wo[:])

                for c in range(SC):
                    qc = qcs[c]
                    if c > 0:
                        for h in range(H):
                            eng = nc.scalar if h % 2 == 0 else nc.sync
                            eng.dma_start(qc[:, h], qt_sp[h][:, c * 512:(c + 1) * 512])
                    nt = 4 * (c + 1)
                    for h in range(H):
                        # attn_outT accumulator [d, sq] and softmax sums [1, sq]
                        ob_ps = opsum.tile([P, 512], f32, tag="obp", name="obp")
                        sm_ps = opsum.tile([1, 512], f32, tag="smp", name="smp")
                        # diagonal tiles first: their exp+mask latency hides
                        # behind the dense unmasked tail of this head and the
                        # previous head's stream
                        t_order = list(range(4 * c, nt)) + list(range(0, 4 * c))
                        for ti, t in enumerate(t_order):
                            r = t - 4 * c
                            off = P * max(r, 0)
                            ps = spsum.tile([P, 512], f32, tag="s")
                            nc.tensor.matmul(
                                ps[:, off:512],
                                kall[:, h, t * P:(t + 1) * P],
                                qc[:, h, off:512],
                                start=True,
                                stop=True,
                            )
                            pt = ppool.tile([P, 512], bf16, tag="pt")
                            nc.scalar.activation(
                                pt[:, off:512], ps[:, off:512], AF.Exp,
                                bias=zb[:], scale=SCALE,
                            )
                            if r >= 0:
                                nc.vector.tensor_mul(
                                    pt[:, off:512], pt[:, off:512], bmt[:, r, off:512]
                                )
                            # P@V with V stationary (one LDWEIGHTS per tile);
                            # output is attn_outT [d, sq] directly
                            nc.tensor.matmul(
                                ob_ps[:, off:512],
                                vsb[:, t, h, 0:128],
                                pt[:, off:512],
                                start=(ti == 0),
                                stop=(ti == nt - 1),
                            )
                            nc.tensor.matmul(
                                sm_ps[:, off:512],
                                ones_col[:],
                                pt[:, off:512],
                                start=(ti == 0),
                                stop=(ti == nt - 1),
                            )
                        # normalize: rcp -> PE broadcast to 128 partitions ->
                        # single DVE multiply into attn_outT SBUF
                        rcp = stage.tile([1, 512], f32, tag="rcp")
                        nc.vector.reciprocal_approx_fast(rcp[:], sm_ps[:])
                        bc_sb = stage.tile([P, 512], f32, tag="bc")
                        nc.gpsimd.partition_broadcast(bc_sb[:], rcp[:])
                        nc.vector.tensor_mul(aot_c[c][:, h], ob_ps[:], bc_sb[:])

                    # o_proj deferred by one chunk: its aot inputs are then
                    # guaranteed ready, so the PE stream never stalls on the
                    # normalize tail
                    if c > 0:
                        emit_oproj(c - 1)
                emit_oproj(SC - 1)

    nc.compile()
    return nc


def _host_prep(hidden_states, position_ids, Wq, Wk, Wv, Wo):
    """Build the 8 per-core input maps."""
    inv_freq = 1.0 / (10000.0 ** (np.arange(0, HD, 2, dtype=np.float32) / HD))
    t = np.arange(S, dtype=np.float32)
    freqs = np.outer(t, inv_freq).astype(np.float32)  # [S, 64]

    bm = np.empty((P, H, 512), dtype=np.float32)
    i = np.arange(P)[:, None, None]
    r = np.arange(H)[None, :, None]
    j = np.arange(512)[None, None, :]
    bm[:] = np.where(i + P * r <= j, 1.0, 0.0)
    bm = bm.astype(ml_dtypes.bfloat16)

    in_maps = []
    per_batch = []
    for b in range(B):
        xT = np.ascontiguousarray(hidden_states[b].T)  # [HID, S]
        xt_sw = np.ascontiguousarray(
            xT.reshape(KO, P, S).transpose(1, 0, 2)
        )  # [P, KO, S]
        fp = freqs[position_ids[b]]  # [S, 64]
        ch = np.cos(fp).T            # [64, S]
        sh = np.sin(fp).T
        cosf = np.ascontiguousarray(np.concatenate([ch, ch], axis=0))   # [128, S]
        sinf = np.ascontiguousarray(np.concatenate([-sh, sh], axis=0))  # signed
        per_batch.append((xt_sw, cosf, sinf))

    for core in range(8):
        b, hg = core // 4, core % 4
        sl = slice(hg * DPC, (hg + 1) * DPC)
        xt_sw, cosf, sinf = per_batch[b]
        wq_sw = np.ascontiguousarray(
            Wq[sl].T.reshape(KO, P, H, HD).transpose(2, 1, 0, 3)
        )  # [H, P, KO, HD]
        wk_sw = np.ascontiguousarray(
            Wk[sl].T.reshape(KO, P, H, HD).transpose(2, 1, 0, 3)
        )
        wv_sw = np.ascontiguousarray(
            Wv[sl].T.reshape(KO, P, DPC).transpose(1, 0, 2)
        )  # [P, KO, DPC]
        wo_sw = np.ascontiguousarray(
            Wo[:, sl].T.reshape(H, HD, HID).transpose(1, 0, 2)
        )  # [P, H, HID]
        in_maps.append({
            "xt": xt_sw, "wq": wq_sw, "wk": wk_sw, "wv": wv_sw, "wo": wo_sw,
            "cosf": cosf, "sinf": sinf, "bmask": bm,
        })
    return in_maps


def kernel(hidden_states, attention_mask, position_ids, Wq, Wk, Wv, Wo,
           _trace=False, _trace_kwargs=None):
    global _CACHED_NC
    hidden_states = np.asarray(hidden_states, dtype=np.float32)
    position_ids = np.asarray(position_ids)
    Wq, Wk, Wv, Wo = (np.asarray(w, dtype=np.float32) for w in (Wq, Wk, Wv, Wo))

    if _CACHED_NC is None:
        _CACHED_NC = build_nc()
    nc = _CACHED_NC

    in_maps = _host_prep(hidden_states, position_ids, Wq, Wk, Wv, Wo)
    res = run_bass_kernel_spmd(
        nc, in_maps, list(range(8)), trace=_trace, **(_trace_kwargs or {})
    )

    out = np.empty((B, S, HID), dtype=np.float32)
    for b in range(B):
        acc = res.results[b * 4]["out_p"].astype(np.float32)
        for hg in range(1, 4):
            acc = acc + res.results[b * 4 + hg]["out_p"]
        out[b] = acc
    if _trace:
        return out, res
    return out



# revision 3
# speedup vs baseline: 1.4366x; 1.4366x over previous
"""TRN2 Bass kernel for causal multi-head attention with RoPE.

Problem: B=2, S=2048, HID=2048, NH=16, HD=128 (fp32 in/out).
Sharding: 8 cores = 2 (batch) x 4 (head-groups of 4 heads).
Each core computes q/k/v projections for its 4 heads (column-parallel),
RoPE, causal attention, and a row-parallel partial o_proj; the host sums
the 4 partials per batch.

v2 (all-bf16 dataflow, no DRAM spill):
  - x / Wq / Wk / Wv / Wo converted to bf16 on host: halves HBM traffic
    and makes every matmul 1 cycle/row regardless of tile width.
  - QT/KT live in SBUF as bf16 (2.1MB each) — the v1 DRAM spill round
    trip (16.8MB) and the chunk-0 reload stall are gone.
  - Q/K weight tiles double-buffered (v1 had 7 x ~6.5us stalls at head
    boundaries, each also dropping the PE p-state clock).
  - Softmax sums accumulate on the idle Vector engine (per-tile adds into
    an f32 accumulator) with a single ones-column matmul per (chunk,head)
    instead of one per tile: removes ~26us of PE rows + 160 LDWEIGHTS.
"""
import os
import sys

if "/opt/trn_rl_repo" not in sys.path:
    sys.path.insert(0, "/opt/trn_rl_repo")

import numpy as np
import ml_dtypes

import concourse.bass as bass
import concourse.mybir as mybir
import concourse.tile as tile
from concourse import bacc
from concourse.bass_utils import run_bass_kernel_spmd
from contextlib import ExitStack

P = 128
B, S, HID, NH = 2, 2048, 2048, 16
HD = HID // NH              # 128
H = 4                       # heads per core
DPC = H * HD                # 512 dims per core
KO = HID // P               # 16 contraction chunks
SC = S // 512               # 4 seq chunks of 512
ST = S // P                 # 16 seq tiles of 128
SCALE = 1.0 / float(np.sqrt(HD))

f32 = mybir.dt.float32
f32r = mybir.dt.float32r
bf16 = mybir.dt.bfloat16

_CACHED_NC = None


def build_nc():
    AF = mybir.ActivationFunctionType
    nc = bacc.Bacc(None, target_bir_lowering=False)

    xt = nc.declare_dram_parameter("xt", [P, KO, S], bf16, isOutput=False)
    wq = nc.declare_dram_parameter("wq", [H, P, KO, HD], bf16, isOutput=False)
    wk = nc.declare_dram_parameter("wk", [H, P, KO, HD], bf16, isOutput=False)
    wv = nc.declare_dram_parameter("wv", [P, KO, DPC], bf16, isOutput=False)
    wo = nc.declare_dram_parameter("wo", [P, H, HID], bf16, isOutput=False)
    cosf = nc.declare_dram_parameter("cosf", [P, S], f32, isOutput=False)
    sinf = nc.declare_dram_parameter("sinf", [P, S], f32, isOutput=False)
    bmask = nc.declare_dram_parameter("bmask", [P, H, 512], bf16, isOutput=False)
    out_p = nc.declare_dram_parameter("out_p", [S, HID], f32, isOutput=True)

    out3 = out_p.rearrange("(st p) n -> p st n", p=P)

    with tile.TileContext(nc) as tc:
        with ExitStack() as top:
            vpool = top.enter_context(tc.tile_pool(name="vpool", bufs=1))
            qkres = top.enter_context(tc.tile_pool(name="qkres", bufs=1))
            const = top.enter_context(tc.tile_pool(name="const", bufs=1))

            vsb = vpool.tile([P, ST, H, 128], bf16)
            # SBUF-resident transposed Q/K: [d, h, s] in bf16
            qt_sb = qkres.tile([P, H, S], bf16)
            kt_sb = qkres.tile([P, H, S], bf16)

            # ---------------- Phase P: projections ----------------
            with ExitStack() as ctx:
                xpool = ctx.enter_context(tc.tile_pool(name="xp", bufs=1))
                wvpool = ctx.enter_context(tc.tile_pool(name="wvp", bufs=1))
                pp = ctx.enter_context(tc.tile_pool(name="pp", bufs=4, space="PSUM"))

                xs = xpool.tile([P, KO, S], bf16)
                wvt = wvpool.tile([P, KO, DPC], bf16)
                # interleave wv/x chunk loads so the first V matmuls can
                # start as soon as (x sc0 ko, wv ko) pairs land
                for ko in range(KO):
                    nc.scalar.dma_start(wvt[:, ko], wv[:, ko])
                    nc.sync.dma_start(xs[:, ko, 0:512], xt[:, ko, 0:512])
                for sc in range(1, SC):
                    for ko in range(KO):
                        eng = nc.sync if (ko % 2 == 0) else nc.scalar
                        eng.dma_start(xs[:, ko, sc * 512:(sc + 1) * 512],
                                      xt[:, ko, sc * 512:(sc + 1) * 512])

                # V natural layout [s, d]: stationary x tile, moving wv
                # (512-wide => full PE rate)
                for st in range(ST):
                    ps = pp.tile([P, 512], f32, tag="vproj")
                    for ko in range(KO):
                        nc.tensor.matmul(
                            ps[:],
                            xs[:, ko, st * P:(st + 1) * P],
                            wvt[:, ko],
                            start=(ko == 0),
                            stop=(ko == KO - 1),
                        )
                    nc.vector.tensor_copy(
                        vsb[:, st],
                        ps.rearrange("p (h d) -> p h d", h=H),
                    )

                cspool = ctx.enter_context(tc.tile_pool(name="cs", bufs=1))
                rtmp = ctx.enter_context(tc.tile_pool(name="rt", bufs=3))
                wpool = ctx.enter_context(tc.tile_pool(name="wqk", bufs=2))
                # full-height tables: cos duplicated halves; sin signed
                # (-sin rows 0:64, +sin rows 64:128) so the combine is one add
                cosT = cspool.tile([P, S], f32)
                sinT = cspool.tile([P, S], f32)
                nc.gpsimd.dma_start(cosT[:], cosf[:])
                nc.gpsimd.dma_start(sinT[:], sinf[:])

                for w4, dst in ((wq, qt_sb), (wk, kt_sb)):
                    for h in range(H):
                        wt = wpool.tile([P, KO, HD], bf16, tag="w")
                        nc.scalar.dma_start(wt[:], w4[h])
                        for sc in range(SC):
                            ssl = slice(sc * 512, (sc + 1) * 512)
                            ps = pp.tile([P, 512], f32, tag="proj")
                            for ko in range(KO):
                                nc.tensor.matmul(
                                    ps[:],
                                    wt[:, ko],
                                    xs[:, ko, ssl],
                                    start=(ko == 0),
                                    stop=(ko == KO - 1),
                                )
                            # RoPE eviction: partition-shifted reads are
                            # legal only with a PSUM operand, so the two
                            # rotate half-ops read ps directly; the combine
                            # writes bf16 into the resident QT/KT.
                            t0 = rtmp.tile([P, 512], f32, tag="t0")
                            t1 = rtmp.tile([P, 512], f32, tag="t1")
                            nc.vector.tensor_mul(t0[0:64], ps[64:128], sinT[0:64, ssl])
                            nc.vector.tensor_mul(t0[64:128], ps[0:64], sinT[64:128, ssl])
                            nc.vector.tensor_mul(t1[:], ps[:], cosT[:, ssl])
                            nc.vector.tensor_add(dst[:, h, ssl], t1[:], t0[:])

            # ------------- Phase A: attention + interleaved o_proj -------------
            with ExitStack() as ctx:
                ppool = ctx.enter_context(tc.tile_pool(name="ppool", bufs=6))
                smpool = ctx.enter_context(tc.tile_pool(name="smp", bufs=2))
                stage = ctx.enter_context(tc.tile_pool(name="stage", bufs=4))
                aopool = ctx.enter_context(tc.tile_pool(name="ao", bufs=1))
                wopool = ctx.enter_context(tc.tile_pool(name="wop", bufs=1))
                ost = ctx.enter_context(tc.tile_pool(name="ost", bufs=4))
                spsum = ctx.enter_context(tc.tile_pool(name="sps", bufs=2, space="PSUM"))
                opsum = ctx.enter_context(tc.tile_pool(name="ops", bufs=2, space="PSUM"))
                opo = ctx.enter_context(tc.tile_pool(name="opo", bufs=2, space="PSUM"))

                bmt = const.tile([P, H, 512], bf16)
                nc.gpsimd.dma_start(bmt[:], bmask[:])
                zb = const.tile([P, 1], f32)
                nc.vector.memset(zb[:], 0.0)
                ones_f = const.tile([P, 1], f32)
                nc.vector.memset(ones_f[:], 1.0)
                # memset can't target f32r; stage via f32 and DVE-copy
                ones_col = const.tile([P, 1], f32r)
                nc.vector.tensor_copy(ones_col[:], ones_f[:])
                wot = wopool.tile([P, H, HID], bf16)
                nc.gpsimd.dma_start(wot[:], wo[:])

                aot_c = [
                    aopool.tile([P, H, 512], bf16, tag=f"aot{c}", name=f"aot{c}")
                    for c in range(SC)
                ]

                def emit_oproj(cc):
                    for st4 in range(4):
                        st = cc * 4 + st4
                        for nch in range(4):
                            pso = opo.tile([P, 512], f32, tag="po", name="pso")
                            for dc in range(H):
                                nc.tensor.matmul(
                                    pso[:],
                                    aot_c[cc][:, dc, st4 * P:(st4 + 1) * P],
                                    wot[:, dc, nch * 512:(nch + 1) * 512],
                                    start=(dc == 0),
                                    stop=(dc == H - 1),
                                )
                            ob = ost.tile([P, 512], f32, tag="ob", name="ob")
                            nc.vector.tensor_copy(ob[:], pso[:])
                            nc.sync.dma_start(
                                out3[:, st, nch * 512:(nch + 1) * 512], ob[:]
                            )

                for c in range(SC):
                    qsl = lambda off: slice(c * 512 + off, (c + 1) * 512)
                    nt = 4 * (c + 1)
                    for h in range(H):
                        # attn_outT accumulator [d, sq] and DVE softmax-sum
                        # accumulator [k mod 128, sq]
                        ob_ps = opsum.tile([P, 512], f32, tag="obp", name="obp")
                        smacc = smpool.tile([P, 512], f32r, tag="sma", name="sma")
                        # diagonal tiles first: their exp+mask latency hides
                        # behind the dense unmasked tail of this head and the
                        # previous head's stream
                        t_order = list(range(4 * c, nt)) + list(range(0, 4 * c))
                        for ti, t in enumerate(t_order):
                            r = t - 4 * c
                            off = P * max(r, 0)
                            ps = spsum.tile([P, 512], f32, tag="s")
                            nc.tensor.matmul(
                                ps[:, off:512],
                                kt_sb[:, h, t * P:(t + 1) * P],
                                qt_sb[:, h, qsl(off)],
                                start=True,
                                stop=True,
                            )
                            pt = ppool.tile([P, 512], bf16, tag="pt")
                            nc.scalar.activation(
                                pt[:, off:512], ps[:, off:512], AF.Exp,
                                bias=zb[:], scale=SCALE,
                            )
                            if r >= 0:
                                nc.vector.tensor_mul(
                                    pt[:, off:512], pt[:, off:512], bmt[:, r, off:512]
                                )
                            # P@V with V stationary; output is attn_outT [d, sq]
                            nc.tensor.matmul(
                                ob_ps[:, off:512],
                                vsb[:, t, h],
                                pt[:, off:512],
                                start=(ti == 0),
                                stop=(ti == nt - 1),
                            )
                            # softmax-sum partials on DVE (off-PE): first tile
                            # is the r=0 diagonal (off=0, full width), so a
                            # copy initializes the whole accumulator
                            if ti == 0:
                                nc.vector.tensor_copy(smacc[:], pt[:])
                            else:
                                nc.vector.tensor_add(
                                    smacc[:, off:512], smacc[:, off:512],
                                    pt[:, off:512],
                                )
                        # single partition-reduce matmul for the sums, then
                        # rcp -> broadcast -> normalize into attn_outT SBUF
                        sm_ps = opsum.tile([1, 512], f32, tag="smp", name="smp")
                        nc.tensor.matmul(
                            sm_ps[:], ones_col[:], smacc[:], start=True, stop=True,
                        )
                        rcp = stage.tile([1, 512], f32, tag="rcp")
                        nc.vector.reciprocal_approx_fast(rcp[:], sm_ps[:])
                        bc_sb = stage.tile([P, 512], f32, tag="bc")
                        nc.gpsimd.partition_broadcast(bc_sb[:], rcp[:])
                        nc.vector.tensor_mul(aot_c[c][:, h], ob_ps[:], bc_sb[:])

                    # o_proj deferred by one chunk: its aot inputs are then
                    # guaranteed ready, so the PE stream never stalls on the
                    # normalize tail
                    if c > 0:
                        emit_oproj(c - 1)
                emit_oproj(SC - 1)

    nc.compile()
    return nc


def _host_prep(hidden_states, position_ids, Wq, Wk, Wv, Wo):
    """Build the 8 per-core input maps (bf16 weights/activations)."""
    inv_freq = 1.0 / (10000.0 ** (np.arange(0, HD, 2, dtype=np.float32) / HD))
    t = np.arange(S, dtype=np.float32)
    freqs = np.outer(t, inv_freq).astype(np.float32)  # [S, 64]

    bm = np.empty((P, H, 512), dtype=np.float32)
    i = np.arange(P)[:, None, None]
    r = np.arange(H)[None, :, None]
    j = np.arange(512)[None, None, :]
    bm[:] = np.where(i + P * r <= j, 1.0, 0.0)
    bm = bm.astype(ml_dtypes.bfloat16)

    in_maps = []
    per_batch = []
    for b in range(B):
        xT = np.ascontiguousarray(hidden_states[b].T)  # [HID, S]
        xt_sw = np.ascontiguousarray(
            xT.reshape(KO, P, S).transpose(1, 0, 2)
        ).astype(ml_dtypes.bfloat16)  # [P, KO, S]
        fp = freqs[position_ids[b]]  # [S, 64]
        ch = np.cos(fp).T            # [64, S]
        sh = np.sin(fp).T
        cosf = np.ascontiguousarray(np.concatenate([ch, ch], axis=0))   # [128, S]
        sinf = np.ascontiguousarray(np.concatenate([-sh, sh], axis=0))  # signed
        per_batch.append((xt_sw, cosf, sinf))

    for core in range(8):
        b, hg = core // 4, core % 4
        sl = slice(hg * DPC, (hg + 1) * DPC)
        xt_sw, cosf, sinf = per_batch[b]
        wq_sw = np.ascontiguousarray(
            Wq[sl].T.reshape(KO, P, H, HD).transpose(2, 1, 0, 3)
        ).astype(ml_dtypes.bfloat16)  # [H, P, KO, HD]
        wk_sw = np.ascontiguousarray(
            Wk[sl].T.reshape(KO, P, H, HD).transpose(2, 1, 0, 3)
        ).astype(ml_dtypes.bfloat16)
        wv_sw = np.ascontiguousarray(
            Wv[sl].T.reshape(KO, P, DPC).transpose(1, 0, 2)
        ).astype(ml_dtypes.bfloat16)  # [P, KO, DPC]
        wo_sw = np.ascontiguousarray(
            Wo[:, sl].T.reshape(H, HD, HID).transpose(1, 0, 2)
        ).astype(ml_dtypes.bfloat16)  # [P, H, HID]
        in_maps.append({
            "xt": xt_sw, "wq": wq_sw, "wk": wk_sw, "wv": wv_sw, "wo": wo_sw,
            "cosf": cosf, "sinf": sinf, "bmask": bm,
        })
    return in_maps


def kernel(hidden_states, attention_mask, position_ids, Wq, Wk, Wv, Wo,
           _trace=False, _trace_kwargs=None):
    global _CACHED_NC
    hidden_states = np.asarray(hidden_states, dtype=np.float32)
    position_ids = np.asarray(position_ids)
    Wq, Wk, Wv, Wo = (np.asarray(w, dtype=np.float32) for w in (Wq, Wk, Wv, Wo))

    if _CACHED_NC is None:
        _CACHED_NC = build_nc()
    nc = _CACHED_NC

    in_maps = _host_prep(hidden_states, position_ids, Wq, Wk, Wv, Wo)
    res = run_bass_kernel_spmd(
        nc, in_maps, list(range(8)), trace=_trace, **(_trace_kwargs or {})
    )

    out = np.empty((B, S, HID), dtype=np.float32)
    for b in range(B):
        acc = res.results[b * 4]["out_p"].astype(np.float32)
        for hg in range(1, 4):
            acc = acc + res.results[b * 4 + hg]["out_p"]
        out[b] = acc
    if _trace:
        return out, res
    return out


# revision 15
# speedup vs baseline: 1.5476x; 1.0773x over previous
"""TRN2 Bass kernel for causal multi-head attention with RoPE.

Problem: B=2, S=2048, HID=2048, NH=16, HD=128 (fp32 in/out).
Sharding: 8 cores = 2 (batch) x 4 (head-groups of 4 heads).
Each core computes q/k/v projections for its 4 heads (column-parallel),
RoPE, causal attention, and a row-parallel partial o_proj; the host sums
the 4 partials per batch.

v2 (all-bf16 dataflow, no DRAM spill):
  - x / Wq / Wk / Wv / Wo converted to bf16 on host: halves HBM traffic
    and makes every matmul 1 cycle/row regardless of tile width.
  - QT/KT live in SBUF as bf16 (2.1MB each) — the v1 DRAM spill round
    trip (16.8MB) and the chunk-0 reload stall are gone.
  - Q/K weight tiles double-buffered (v1 had 7 x ~6.5us stalls at head
    boundaries, each also dropping the PE p-state clock).
  - Softmax sums accumulate on the idle Vector engine (per-tile adds into
    an f32 accumulator) with a single ones-column matmul per (chunk,head)
    instead of one per tile: removes ~26us of PE rows + 160 LDWEIGHTS.
"""
import os
import sys

if "/opt/trn_rl_repo" not in sys.path:
    sys.path.insert(0, "/opt/trn_rl_repo")

import numpy as np
import ml_dtypes

import concourse.bass as bass
import concourse.mybir as mybir
import concourse.tile as tile
from concourse import bacc
from concourse.bass_utils import run_bass_kernel_spmd
from contextlib import ExitStack

P = 128
B, S, HID, NH = 2, 2048, 2048, 16
HD = HID // NH              # 128
H = 4                       # heads per core
DPC = H * HD                # 512 dims per core
KO = HID // P               # 16 contraction chunks
SC = S // 512               # 4 seq chunks of 512
ST = S // P                 # 16 seq tiles of 128
SCALE = 1.0 / float(np.sqrt(HD))

f32 = mybir.dt.float32
f32r = mybir.dt.float32r
bf16 = mybir.dt.bfloat16
fp16 = mybir.dt.float16

_CACHED_NC = None


def build_nc():
    AF = mybir.ActivationFunctionType
    nc = bacc.Bacc(None, target_bir_lowering=False)

    xt = nc.declare_dram_parameter("xt", [P, KO, S], bf16, isOutput=False)
    wq = nc.declare_dram_parameter("wq", [H, P, KO, HD], bf16, isOutput=False)
    wk = nc.declare_dram_parameter("wk", [H, P, KO, HD], bf16, isOutput=False)
    wv = nc.declare_dram_parameter("wv", [P, KO, DPC], bf16, isOutput=False)
    wo = nc.declare_dram_parameter("wo", [P, H, HID], bf16, isOutput=False)
    cosf = nc.declare_dram_parameter("cosf", [P, S], f32, isOutput=False)
    sinf = nc.declare_dram_parameter("sinf", [P, S], f32, isOutput=False)
    bmask = nc.declare_dram_parameter("bmask", [P, H, 512], fp16, isOutput=False)
    out_p = nc.declare_dram_parameter("out_p", [S, HID], f32, isOutput=True)

    out3 = out_p.rearrange("(st p) n -> p st n", p=P)

    with tile.TileContext(nc) as tc:
        with ExitStack() as top:
            vpool = top.enter_context(tc.tile_pool(name="vpool", bufs=1))
            qkres = top.enter_context(tc.tile_pool(name="qkres", bufs=1))
            const = top.enter_context(tc.tile_pool(name="const", bufs=1))

            vsb = vpool.tile([P, ST, H, 128], fp16)
            # SBUF-resident transposed Q/K: [d, h, s] in bf16
            qt_sb = qkres.tile([P, H, S], bf16)
            kt_sb = qkres.tile([P, H, S], bf16)

            zb = const.tile([P, 1], f32)
            nc.vector.memset(zb[:], 0.0)
            # warm the scalar-engine exp table so the first attention tile
            # doesn't eat the ACT_TABLE_LOAD latency
            warm = const.tile([P, 1], fp16)
            nc.scalar.activation(warm[:], zb[:], AF.Exp, bias=zb[:], scale=1.0)

            # ---------------- Phase P: projections ----------------
            with ExitStack() as ctx:
                xpool = ctx.enter_context(tc.tile_pool(name="xp", bufs=1))
                wvpool = ctx.enter_context(tc.tile_pool(name="wvp", bufs=1))
                pp = ctx.enter_context(tc.tile_pool(name="pp", bufs=4, space="PSUM"))

                # per-chunk x tiles + quarter wv tiles: Tile dependencies are
                # tile-granular, so finer tiles let the first V matmuls start
                # after ~2.6MB instead of after the whole stream. wv quarters
                # land first (0.5MB each), then the x chunk halves.
                xsc = [xpool.tile([P, KO, 512], bf16, tag=f"xs{sc}", name=f"xs{sc}")
                       for sc in range(SC)]
                wvq = [wvpool.tile([P, KO // 4, DPC], bf16, tag=f"wv{j}",
                                   name=f"wv{j}") for j in range(4)]
                nc.sync.dma_start(wvq[0][:], wv[:, 0:4])
                nc.scalar.dma_start(wvq[1][:], wv[:, 4:8])
                for sc in range(SC):
                    ssl = slice(sc * 512, (sc + 1) * 512)
                    nc.sync.dma_start(xsc[sc][:, 0:8], xt[:, 0:8, ssl])
                    nc.scalar.dma_start(xsc[sc][:, 8:16], xt[:, 8:16, ssl])
                    if sc == 0:
                        nc.sync.dma_start(wvq[2][:], wv[:, 8:12])
                        nc.scalar.dma_start(wvq[3][:], wv[:, 12:16])

                # V natural layout [s, d]: stationary x tile, moving wv
                # (512-wide => full PE rate)
                for st in range(ST):
                    xc = xsc[st // 4]
                    so = (st % 4) * P
                    ps = pp.tile([P, 512], f32, tag="vproj")
                    for ko in range(KO):
                        wvm = wvq[ko // 4][:, ko % 4]
                        nc.tensor.matmul(
                            ps[:],
                            xc[:, ko, so:so + P],
                            wvm,
                            start=(ko == 0),
                            stop=(ko == KO - 1),
                        )
                    nc.vector.tensor_copy(
                        vsb[:, st],
                        ps.rearrange("p (h d) -> p h d", h=H),
                    )

                cspool = ctx.enter_context(tc.tile_pool(name="cs", bufs=1))
                rtmp = ctx.enter_context(tc.tile_pool(name="rt", bufs=3))
                wpool = ctx.enter_context(tc.tile_pool(name="wqk", bufs=2))
                # full-height tables: cos duplicated halves; sin signed
                # (-sin rows 0:64, +sin rows 64:128) so the combine is one add
                cosT = cspool.tile([P, S], f32)
                sinT = cspool.tile([P, S], f32)
                nc.gpsimd.dma_start(cosT[:], cosf[:])
                nc.gpsimd.dma_start(sinT[:], sinf[:])

                for w4, dst in ((wq, qt_sb), (wk, kt_sb)):
                    for h in range(H):
                        wt = wpool.tile([P, KO, HD], bf16, tag="w")
                        nc.scalar.dma_start(wt[:], w4[h])
                        for sc in range(SC):
                            ssl = slice(sc * 512, (sc + 1) * 512)
                            ps = pp.tile([P, 512], f32, tag="proj")
                            for ko in range(KO):
                                nc.tensor.matmul(
                                    ps[:],
                                    wt[:, ko],
                                    xsc[sc][:, ko],
                                    start=(ko == 0),
                                    stop=(ko == KO - 1),
                                )
                            # RoPE eviction: partition-shifted reads are
                            # legal only with a PSUM operand, so the two
                            # rotate half-ops read ps directly; the combine
                            # writes bf16 into the resident QT/KT.
                            t0 = rtmp.tile([P, 512], f32, tag="t0")
                            t1 = rtmp.tile([P, 512], f32, tag="t1")
                            nc.vector.tensor_mul(t0[0:64], ps[64:128], sinT[0:64, ssl])
                            nc.vector.tensor_mul(t0[64:128], ps[0:64], sinT[64:128, ssl])
                            nc.vector.tensor_mul(t1[:], ps[:], cosT[:, ssl])
                            nc.vector.tensor_add(dst[:, h, ssl], t1[:], t0[:])

            # ------------- Phase A: attention + interleaved o_proj -------------
            with ExitStack() as ctx:
                ppool = ctx.enter_context(tc.tile_pool(name="ppool", bufs=6))
                smpool = ctx.enter_context(tc.tile_pool(name="smp", bufs=2))
                stage = ctx.enter_context(tc.tile_pool(name="stage", bufs=4))
                aopool = ctx.enter_context(tc.tile_pool(name="ao", bufs=1))
                wopool = ctx.enter_context(tc.tile_pool(name="wop", bufs=1))
                ost = ctx.enter_context(tc.tile_pool(name="ost", bufs=4))
                spsum = ctx.enter_context(tc.tile_pool(name="sps", bufs=2, space="PSUM"))
                opsum = ctx.enter_context(tc.tile_pool(name="ops", bufs=2, space="PSUM"))
                smps = ctx.enter_context(tc.tile_pool(name="smps", bufs=1, space="PSUM"))
                opo = ctx.enter_context(tc.tile_pool(name="opo", bufs=3, space="PSUM"))

                bmt = const.tile([P, H, 512], fp16)
                nc.gpsimd.dma_start(bmt[:], bmask[:])
                ones_col = const.tile([P, 1], fp16)
                nc.vector.memset(ones_col[:], 1.0)
                wot = wopool.tile([P, H, HID], bf16)
                nc.gpsimd.dma_start(wot[:], wo[:])

                aot_c = [
                    aopool.tile([P, H, 512], bf16, tag=f"aot{c}", name=f"aot{c}")
                    for c in range(SC)
                ]

                def emit_oproj(cc):
                    for st4 in range(4):
                        st = cc * 4 + st4
                        for nch in range(4):
                            g = st4 * 4 + nch
                            pso = opo.tile([P, 512], f32, tag="po", name="pso")
                            for dc in range(H):
                                nc.tensor.matmul(
                                    pso[:],
                                    aot_c[cc][:, dc, st4 * P:(st4 + 1) * P],
                                    wot[:, dc, nch * 512:(nch + 1) * 512],
                                    start=(dc == 0),
                                    stop=(dc == H - 1),
                                )
                            # PSUM->SBUF eviction split between scalar ACT
                            # and DVE (gpsimd cannot read PSUM)
                            ob = ost.tile([P, 512], f32, tag="ob", name="ob")
                            if g % 2 == 0:
                                nc.scalar.activation(ob[:], pso[:], AF.Copy)
                            else:
                                nc.vector.tensor_copy(ob[:], pso[:])
                            eng = nc.sync if g % 2 == 0 else nc.gpsimd
                            eng.dma_start(
                                out3[:, st, nch * 512:(nch + 1) * 512], ob[:]
                            )

                for c in range(SC):
                    qsl = lambda off: slice(c * 512 + off, (c + 1) * 512)
                    nt = 4 * (c + 1)
                    for h in range(H):
                        # attn_outT accumulator [d, sq] and DVE softmax-sum
                        # accumulator [k mod 128, sq]
                        ob_ps = opsum.tile([P, 512], f32, tag="obp", name="obp")
                        smacc = smpool.tile([P, 512], fp16, tag="sma", name="sma")
                        # diagonal tiles first: their exp+mask latency hides
                        # behind the dense unmasked tail of this head and the
                        # previous head's stream
                        t_order = list(range(4 * c, nt)) + list(range(0, 4 * c))
                        for ti, t in enumerate(t_order):
                            r = t - 4 * c
                            off = P * max(r, 0)
                            ps = spsum.tile([P, 512], f32, tag="s")
                            nc.tensor.matmul(
                                ps[:, off:512],
                                kt_sb[:, h, t * P:(t + 1) * P],
                                qt_sb[:, h, qsl(off)],
                                start=True,
                                stop=True,
                            )
                            pt = ppool.tile([P, 512], fp16, tag="pt")
                            nc.scalar.activation(
                                pt[:, off:512], ps[:, off:512], AF.Exp,
                                bias=zb[:], scale=SCALE,
                            )
                            if r >= 0:
                                nc.vector.tensor_mul(
                                    pt[:, off:512], pt[:, off:512], bmt[:, r, off:512]
                                )
                            # P@V with V stationary; output is attn_outT [d, sq]
                            nc.tensor.matmul(
                                ob_ps[:, off:512],
                                vsb[:, t, h],
                                pt[:, off:512],
                                start=(ti == 0),
                                stop=(ti == nt - 1),
                            )
                            # softmax-sum partials on DVE (off-PE): first tile
                            # is the r=0 diagonal (off=0, full width), so a
                            # copy initializes the whole accumulator
                            if ti == 0:
                                nc.vector.tensor_copy(smacc[:], pt[:])
                            else:
                                nc.vector.tensor_add(
                                    smacc[:, off:512], smacc[:, off:512],
                                    pt[:, off:512],
                                )
                        # single partition-reduce matmul for the sums, then
                        # rcp -> broadcast -> normalize into attn_outT SBUF
                        sm_ps = smps.tile([1, 512], f32, tag="smp", name="smp")
                        nc.tensor.matmul(
                            sm_ps[:], ones_col[:], smacc[:], start=True, stop=True,
                        )
                        rcp = stage.tile([1, 512], f32, tag="rcp")
                        nc.vector.reciprocal_approx_fast(rcp[:], sm_ps[:])
                        bc_sb = stage.tile([P, 512], f32, tag="bc")
                        nc.gpsimd.partition_broadcast(bc_sb[:], rcp[:])
                        nc.vector.tensor_mul(aot_c[c][:, h], ob_ps[:], bc_sb[:])

                    # o_proj deferred by one chunk: its aot inputs are then
                    # guaranteed ready, so the PE stream never stalls on the
                    # normalize tail
                    if c > 0:
                        emit_oproj(c - 1)
                emit_oproj(SC - 1)

    nc.compile()
    return nc


def _host_prep(hidden_states, position_ids, Wq, Wk, Wv, Wo):
    """Build the 8 per-core input maps (bf16 weights/activations)."""
    inv_freq = 1.0 / (10000.0 ** (np.arange(0, HD, 2, dtype=np.float32) / HD))
    t = np.arange(S, dtype=np.float32)
    freqs = np.outer(t, inv_freq).astype(np.float32)  # [S, 64]

    bm = np.empty((P, H, 512), dtype=np.float32)
    i = np.arange(P)[:, None, None]
    r = np.arange(H)[None, :, None]
    j = np.arange(512)[None, None, :]
    bm[:] = np.where(i + P * r <= j, 1.0, 0.0)
    bm = bm.astype(np.float16)

    in_maps = []
    per_batch = []
    for b in range(B):
        xT = np.ascontiguousarray(hidden_states[b].T)  # [HID, S]
        xt_sw = np.ascontiguousarray(
            xT.reshape(KO, P, S).transpose(1, 0, 2)
        ).astype(ml_dtypes.bfloat16)  # [P, KO, S]
        fp = freqs[position_ids[b]]  # [S, 64]
        ch = np.cos(fp).T            # [64, S]
        sh = np.sin(fp).T
        cosf = np.ascontiguousarray(np.concatenate([ch, ch], axis=0))   # [128, S]
        sinf = np.ascontiguousarray(np.concatenate([-sh, sh], axis=0))  # signed
        per_batch.append((xt_sw, cosf, sinf))

    for core in range(8):
        b, hg = core // 4, core % 4
        sl = slice(hg * DPC, (hg + 1) * DPC)
        xt_sw, cosf, sinf = per_batch[b]
        wq_sw = np.ascontiguousarray(
            Wq[sl].T.reshape(KO, P, H, HD).transpose(2, 1, 0, 3)
        ).astype(ml_dtypes.bfloat16)  # [H, P, KO, HD]
        wk_sw = np.ascontiguousarray(
            Wk[sl].T.reshape(KO, P, H, HD).transpose(2, 1, 0, 3)
        ).astype(ml_dtypes.bfloat16)
        wv_sw = np.ascontiguousarray(
            Wv[sl].T.reshape(KO, P, DPC).transpose(1, 0, 2)
        ).astype(ml_dtypes.bfloat16)  # [P, KO, DPC]
        wo_sw = np.ascontiguousarray(
            Wo[:, sl].T.reshape(H, HD, HID).transpose(1, 0, 2)
        ).astype(ml_dtypes.bfloat16)  # [P, H, HID]
        in_maps.append({
            "xt": xt_sw, "wq": wq_sw, "wk": wk_sw, "wv": wv_sw, "wo": wo_sw,
            "cosf": cosf, "sinf": sinf, "bmask": bm,
        })
    return in_maps


def kernel(hidden_states, attention_mask, position_ids, Wq, Wk, Wv, Wo,
           _trace=False, _trace_kwargs=None):
    global _CACHED_NC
    hidden_states = np.asarray(hidden_states, dtype=np.float32)
    position_ids = np.asarray(position_ids)
    Wq, Wk, Wv, Wo = (np.asarray(w, dtype=np.float32) for w in (Wq, Wk, Wv, Wo))

    if _CACHED_NC is None:
        _CACHED_NC = build_nc()
    nc = _CACHED_NC

    in_maps = _host_prep(hidden_states, position_ids, Wq, Wk, Wv, Wo)
    res = run_bass_kernel_spmd(
        nc, in_maps, list(range(8)), trace=_trace, **(_trace_kwargs or {})
    )

    out = np.empty((B, S, HID), dtype=np.float32)
    for b in range(B):
        acc = res.results[b * 4]["out_p"].astype(np.float32)
        for hg in range(1, 4):
            acc = acc + res.results[b * 4 + hg]["out_p"]
        out[b] = acc
    if _trace:
        return out, res
    return out


# revision 21
# speedup vs baseline: 1.5699x; 1.0144x over previous
"""TRN2 Bass kernel for causal multi-head attention with RoPE.

Problem: B=2, S=2048, HID=2048, NH=16, HD=128 (fp32 in/out).
Sharding: 8 cores = 2 (batch) x 4 (head-groups of 4 heads).
Each core computes q/k/v projections for its 4 heads (column-parallel),
RoPE, causal attention, and a row-parallel partial o_proj; the host sums
the 4 partials per batch.

v2 (all-bf16 dataflow, no DRAM spill):
  - x / Wq / Wk / Wv / Wo converted to bf16 on host: halves HBM traffic
    and makes every matmul 1 cycle/row regardless of tile width.
  - QT/KT live in SBUF as bf16 (2.1MB each) — the v1 DRAM spill round
    trip (16.8MB) and the chunk-0 reload stall are gone.
  - Q/K weight tiles double-buffered (v1 had 7 x ~6.5us stalls at head
    boundaries, each also dropping the PE p-state clock).
  - Softmax sums accumulate on the idle Vector engine (per-tile adds into
    an f32 accumulator) with a single ones-column matmul per (chunk,head)
    instead of one per tile: removes ~26us of PE rows + 160 LDWEIGHTS.
"""
import os
import sys

if "/opt/trn_rl_repo" not in sys.path:
    sys.path.insert(0, "/opt/trn_rl_repo")

import numpy as np
import ml_dtypes

import concourse.bass as bass
import concourse.mybir as mybir
import concourse.tile as tile
from concourse import bacc
from concourse.bass_utils import run_bass_kernel_spmd
from contextlib import ExitStack

P = 128
B, S, HID, NH = 2, 2048, 2048, 16
HD = HID // NH              # 128
H = 4                       # heads per core
DPC = H * HD                # 512 dims per core
KO = HID // P               # 16 contraction chunks
SC = S // 512               # 4 seq chunks of 512
ST = S // P                 # 16 seq tiles of 128
SCALE = 1.0 / float(np.sqrt(HD))

f32 = mybir.dt.float32
f32r = mybir.dt.float32r
bf16 = mybir.dt.bfloat16
fp16 = mybir.dt.float16

_CACHED_NC = None


def build_nc():
    AF = mybir.ActivationFunctionType
    nc = bacc.Bacc(None, target_bir_lowering=False)

    xt = nc.declare_dram_parameter("xt", [P, KO, S], bf16, isOutput=False)
    wq = nc.declare_dram_parameter("wq", [H, P, KO, HD], bf16, isOutput=False)
    wk = nc.declare_dram_parameter("wk", [H, P, KO, HD], bf16, isOutput=False)
    wv = nc.declare_dram_parameter("wv", [P, KO, DPC], bf16, isOutput=False)
    wo = nc.declare_dram_parameter("wo", [P, H, HID], bf16, isOutput=False)
    cosf = nc.declare_dram_parameter("cosf", [P, S], f32, isOutput=False)
    sinf = nc.declare_dram_parameter("sinf", [P, S], f32, isOutput=False)
    bmask = nc.declare_dram_parameter("bmask", [P, H, 512], fp16, isOutput=False)
    # bf16 partials: host sums the 4 head-group partials in f32
    out_p = nc.declare_dram_parameter("out_p", [S, HID], bf16, isOutput=True)

    out3 = out_p.rearrange("(st p) n -> p st n", p=P)

    with tile.TileContext(nc) as tc:
        with ExitStack() as top:
            vpool = top.enter_context(tc.tile_pool(name="vpool", bufs=1))
            qkres = top.enter_context(tc.tile_pool(name="qkres", bufs=1))
            const = top.enter_context(tc.tile_pool(name="const", bufs=1))

            vsb = vpool.tile([P, ST, H, 128], fp16)
            # SBUF-resident transposed Q/K: [d, h, s] in bf16
            qt_sb = qkres.tile([P, H, S], bf16)
            kt_sb = qkres.tile([P, H, S], bf16)

            zb = const.tile([P, 1], f32)
            nc.vector.memset(zb[:], 0.0)
            # warm the scalar-engine exp table so the first attention tile
            # doesn't eat the ACT_TABLE_LOAD latency
            warm = const.tile([P, 1], fp16)
            nc.scalar.activation(warm[:], zb[:], AF.Exp, bias=zb[:], scale=1.0)

            # ---------------- Phase P: projections ----------------
            with ExitStack() as ctx:
                xpool = ctx.enter_context(tc.tile_pool(name="xp", bufs=1))
                wvpool = ctx.enter_context(tc.tile_pool(name="wvp", bufs=1))
                pp = ctx.enter_context(tc.tile_pool(name="pp", bufs=4, space="PSUM"))

                # per-chunk x tiles + quarter wv tiles: Tile dependencies are
                # tile-granular, so finer tiles let the first V matmuls start
                # after ~2.6MB instead of after the whole stream. wv quarters
                # land first (0.5MB each), then the x chunk halves.
                xsc = [xpool.tile([P, KO, 512], bf16, tag=f"xs{sc}", name=f"xs{sc}")
                       for sc in range(SC)]
                wvq = [wvpool.tile([P, KO // 4, DPC], bf16, tag=f"wv{j}",
                                   name=f"wv{j}") for j in range(4)]
                # DMA bandwidth ramps from ~130GB/s over the first ~20us, so
                # the critical first 2.6MB (x chunk 0 + wv) rides all three
                # queues in parallel; everything else queues behind it.
                nc.sync.dma_start(wvq[0][:], wv[:, 0:4])
                nc.scalar.dma_start(wvq[1][:], wv[:, 4:8])
                nc.gpsimd.dma_start(wvq[2][:], wv[:, 8:12])
                nc.sync.dma_start(xsc[0][:, 0:6], xt[:, 0:6, 0:512])
                nc.scalar.dma_start(xsc[0][:, 6:11], xt[:, 6:11, 0:512])
                nc.gpsimd.dma_start(xsc[0][:, 11:16], xt[:, 11:16, 0:512])
                nc.sync.dma_start(wvq[3][:], wv[:, 12:16])
                for sc in range(1, SC):
                    ssl = slice(sc * 512, (sc + 1) * 512)
                    nc.sync.dma_start(xsc[sc][:, 0:8], xt[:, 0:8, ssl])
                    nc.scalar.dma_start(xsc[sc][:, 8:16], xt[:, 8:16, ssl])

                # V natural layout [s, d]: stationary x tile, moving wv
                # (512-wide => full PE rate)
                for st in range(ST):
                    xc = xsc[st // 4]
                    so = (st % 4) * P
                    ps = pp.tile([P, 512], f32, tag="vproj")
                    for ko in range(KO):
                        wvm = wvq[ko // 4][:, ko % 4]
                        nc.tensor.matmul(
                            ps[:],
                            xc[:, ko, so:so + P],
                            wvm,
                            start=(ko == 0),
                            stop=(ko == KO - 1),
                        )
                    nc.vector.tensor_copy(
                        vsb[:, st],
                        ps.rearrange("p (h d) -> p h d", h=H),
                    )

                cspool = ctx.enter_context(tc.tile_pool(name="cs", bufs=1))
                rtmp = ctx.enter_context(tc.tile_pool(name="rt", bufs=3))
                wpool = ctx.enter_context(tc.tile_pool(name="wqk", bufs=2))
                # full-height tables: cos duplicated halves; sin signed
                # (-sin rows 0:64, +sin rows 64:128) so the combine is one add
                cosT = cspool.tile([P, S], f32)
                sinT = cspool.tile([P, S], f32)
                nc.gpsimd.dma_start(cosT[:], cosf[:])
                nc.gpsimd.dma_start(sinT[:], sinf[:])

                for w4, dst in ((wq, qt_sb), (wk, kt_sb)):
                    for h in range(H):
                        wt = wpool.tile([P, KO, HD], bf16, tag="w")
                        nc.scalar.dma_start(wt[:], w4[h])
                        for sc in range(SC):
                            ssl = slice(sc * 512, (sc + 1) * 512)
                            ps = pp.tile([P, 512], f32, tag="proj")
                            for ko in range(KO):
                                nc.tensor.matmul(
                                    ps[:],
                                    wt[:, ko],
                                    xsc[sc][:, ko],
                                    start=(ko == 0),
                                    stop=(ko == KO - 1),
                                )
                            # RoPE eviction: partition-shifted reads are
                            # legal only with a PSUM operand, so the two
                            # rotate half-ops read ps directly; the combine
                            # writes bf16 into the resident QT/KT.
                            t0 = rtmp.tile([P, 512], f32, tag="t0")
                            t1 = rtmp.tile([P, 512], f32, tag="t1")
                            nc.vector.tensor_mul(t0[0:64], ps[64:128], sinT[0:64, ssl])
                            nc.vector.tensor_mul(t0[64:128], ps[0:64], sinT[64:128, ssl])
                            nc.vector.tensor_mul(t1[:], ps[:], cosT[:, ssl])
                            nc.vector.tensor_add(dst[:, h, ssl], t1[:], t0[:])

            # ------------- Phase A: attention + interleaved o_proj -------------
            with ExitStack() as ctx:
                ppool = ctx.enter_context(tc.tile_pool(name="ppool", bufs=6))
                smpool = ctx.enter_context(tc.tile_pool(name="smp", bufs=2))
                stage = ctx.enter_context(tc.tile_pool(name="stage", bufs=4))
                aopool = ctx.enter_context(tc.tile_pool(name="ao", bufs=1))
                wopool = ctx.enter_context(tc.tile_pool(name="wop", bufs=1))
                ost = ctx.enter_context(tc.tile_pool(name="ost", bufs=4))
                spsum = ctx.enter_context(tc.tile_pool(name="sps", bufs=2, space="PSUM"))
                opsum = ctx.enter_context(tc.tile_pool(name="ops", bufs=2, space="PSUM"))
                smps = ctx.enter_context(tc.tile_pool(name="smps", bufs=1, space="PSUM"))
                opo = ctx.enter_context(tc.tile_pool(name="opo", bufs=3, space="PSUM"))

                bmt = const.tile([P, H, 512], fp16)
                nc.gpsimd.dma_start(bmt[:], bmask[:])
                ones_col = const.tile([P, 1], fp16)
                nc.vector.memset(ones_col[:], 1.0)
                wot = wopool.tile([P, H, HID], bf16)
                nc.gpsimd.dma_start(wot[:], wo[:])

                aot_c = [
                    aopool.tile([P, H, 512], bf16, tag=f"aot{c}", name=f"aot{c}")
                    for c in range(SC)
                ]

                def emit_oproj(cc):
                    for st4 in range(4):
                        st = cc * 4 + st4
                        for nch in range(4):
                            g = st4 * 4 + nch
                            pso = opo.tile([P, 512], f32, tag="po", name="pso")
                            for dc in range(H):
                                nc.tensor.matmul(
                                    pso[:],
                                    aot_c[cc][:, dc, st4 * P:(st4 + 1) * P],
                                    wot[:, dc, nch * 512:(nch + 1) * 512],
                                    start=(dc == 0),
                                    stop=(dc == H - 1),
                                )
                            # PSUM->SBUF eviction split between scalar ACT
                            # and DVE (gpsimd cannot read PSUM); bf16 out
                            # halves the write stream, spread over 3 queues
                            ob = ost.tile([P, 512], bf16, tag="ob", name="ob")
                            if g % 2 == 0:
                                nc.scalar.activation(ob[:], pso[:], AF.Copy)
                            else:
                                nc.vector.tensor_copy(ob[:], pso[:])
                            eng = (nc.sync, nc.gpsimd, nc.scalar)[g % 3]
                            eng.dma_start(
                                out3[:, st, nch * 512:(nch + 1) * 512], ob[:]
                            )

                # the per-head normalize chain (sm matmul -> rcp -> gpsimd
                # broadcast -> DVE mul) is deferred by one head: issued
                # immediately it sits at the head of the in-order DVE queue
                # waiting on gpsimd and blocks the next head's mask/sum ops
                pending = []

                def norm_flush():
                    if not pending:
                        return
                    pc, ph, pob, psm = pending.pop()
                    sm_ps = smps.tile([1, 512], f32, tag="smp", name="smp")
                    nc.tensor.matmul(
                        sm_ps[:], ones_col[:], psm[:], start=True, stop=True,
                    )
                    rcp = stage.tile([1, 512], f32, tag="rcp")
                    nc.vector.reciprocal_approx_fast(rcp[:], sm_ps[:])
                    bc_sb = stage.tile([P, 512], f32, tag="bc")
                    nc.gpsimd.partition_broadcast(bc_sb[:], rcp[:])
                    nc.vector.tensor_mul(aot_c[pc][:, ph], pob[:], bc_sb[:])

                for c in range(SC):
                    qsl = lambda off: slice(c * 512 + off, (c + 1) * 512)
                    nt = 4 * (c + 1)
                    for h in range(H):
                        # attn_outT accumulator [d, sq] and DVE softmax-sum
                        # accumulator [k mod 128, sq]
                        ob_ps = opsum.tile([P, 512], f32, tag="obp", name="obp")
                        smacc = smpool.tile([P, 512], fp16, tag="sma", name="sma")
                        # diagonal tiles first: their exp+mask latency hides
                        # behind the dense unmasked tail of this head and the
                        # previous head's stream
                        t_order = list(range(4 * c, nt)) + list(range(0, 4 * c))
                        for ti, t in enumerate(t_order):
                            r = t - 4 * c
                            off = P * max(r, 0)
                            ps = spsum.tile([P, 512], f32, tag="s")
                            nc.tensor.matmul(
                                ps[:, off:512],
                                kt_sb[:, h, t * P:(t + 1) * P],
                                qt_sb[:, h, qsl(off)],
                                start=True,
                                stop=True,
                            )
                            pt = ppool.tile([P, 512], fp16, tag="pt")
                            nc.scalar.activation(
                                pt[:, off:512], ps[:, off:512], AF.Exp,
                                bias=zb[:], scale=SCALE,
                            )
                            if r >= 0:
                                nc.vector.tensor_mul(
                                    pt[:, off:512], pt[:, off:512], bmt[:, r, off:512]
                                )
                            # P@V with V stationary; output is attn_outT [d, sq]
                            nc.tensor.matmul(
                                ob_ps[:, off:512],
                                vsb[:, t, h],
                                pt[:, off:512],
                                start=(ti == 0),
                                stop=(ti == nt - 1),
                            )
                            # softmax-sum partials on DVE (off-PE): first tile
                            # is the r=0 diagonal (off=0, full width), so a
                            # copy initializes the whole accumulator
                            if ti == 0:
                                nc.vector.tensor_copy(smacc[:], pt[:])
                            else:
                                nc.vector.tensor_add(
                                    smacc[:, off:512], smacc[:, off:512],
                                    pt[:, off:512],
                                )
                        # normalize the PREVIOUS head now; queue this one
                        norm_flush()
                        pending.append((c, h, ob_ps, smacc))

                    # o_proj deferred by one chunk: its aot inputs are then
                    # guaranteed ready, so the PE stream never stalls on the
                    # normalize tail
                    if c > 0:
                        emit_oproj(c - 1)
                norm_flush()
                emit_oproj(SC - 1)

    nc.compile()
    return nc


def _host_prep(hidden_states, position_ids, Wq, Wk, Wv, Wo):
    """Build the 8 per-core input maps (bf16 weights/activations)."""
    inv_freq = 1.0 / (10000.0 ** (np.arange(0, HD, 2, dtype=np.float32) / HD))
    t = np.arange(S, dtype=np.float32)
    freqs = np.outer(t, inv_freq).astype(np.float32)  # [S, 64]

    bm = np.empty((P, H, 512), dtype=np.float32)
    i = np.arange(P)[:, None, None]
    r = np.arange(H)[None, :, None]
    j = np.arange(512)[None, None, :]
    bm[:] = np.where(i + P * r <= j, 1.0, 0.0)
    bm = bm.astype(np.float16)

    in_maps = []
    per_batch = []
    for b in range(B):
        xT = np.ascontiguousarray(hidden_states[b].T)  # [HID, S]
        xt_sw = np.ascontiguousarray(
            xT.reshape(KO, P, S).transpose(1, 0, 2)
        ).astype(ml_dtypes.bfloat16)  # [P, KO, S]
        fp = freqs[position_ids[b]]  # [S, 64]
        ch = np.cos(fp).T            # [64, S]
        sh = np.sin(fp).T
        cosf = np.ascontiguousarray(np.concatenate([ch, ch], axis=0))   # [128, S]
        sinf = np.ascontiguousarray(np.concatenate([-sh, sh], axis=0))  # signed
        per_batch.append((xt_sw, cosf, sinf))

    for core in range(8):
        b, hg = core // 4, core % 4
        sl = slice(hg * DPC, (hg + 1) * DPC)
        xt_sw, cosf, sinf = per_batch[b]
        wq_sw = np.ascontiguousarray(
            Wq[sl].T.reshape(KO, P, H, HD).transpose(2, 1, 0, 3)
        ).astype(ml_dtypes.bfloat16)  # [H, P, KO, HD]
        wk_sw = np.ascontiguousarray(
            Wk[sl].T.reshape(KO, P, H, HD).transpose(2, 1, 0, 3)
        ).astype(ml_dtypes.bfloat16)
        wv_sw = np.ascontiguousarray(
            Wv[sl].T.reshape(KO, P, DPC).transpose(1, 0, 2)
        ).astype(ml_dtypes.bfloat16)  # [P, KO, DPC]
        wo_sw = np.ascontiguousarray(
            Wo[:, sl].T.reshape(H, HD, HID).transpose(1, 0, 2)
        ).astype(ml_dtypes.bfloat16)  # [P, H, HID]
        in_maps.append({
            "xt": xt_sw, "wq": wq_sw, "wk": wk_sw, "wv": wv_sw, "wo": wo_sw,
            "cosf": cosf, "sinf": sinf, "bmask": bm,
        })
    return in_maps


def kernel(hidden_states, attention_mask, position_ids, Wq, Wk, Wv, Wo,
           _trace=False, _trace_kwargs=None):
    global _CACHED_NC
    hidden_states = np.asarray(hidden_states, dtype=np.float32)
    position_ids = np.asarray(position_ids)
    Wq, Wk, Wv, Wo = (np.asarray(w, dtype=np.float32) for w in (Wq, Wk, Wv, Wo))

    if _CACHED_NC is None:
        _CACHED_NC = build_nc()
    nc = _CACHED_NC

    in_maps = _host_prep(hidden_states, position_ids, Wq, Wk, Wv, Wo)
    res = run_bass_kernel_spmd(
        nc, in_maps, list(range(8)), trace=_trace, **(_trace_kwargs or {})
    )

    out = np.empty((B, S, HID), dtype=np.float32)
    for b in range(B):
        acc = res.results[b * 4]["out_p"].astype(np.float32)
        for hg in range(1, 4):
            acc = acc + res.results[b * 4 + hg]["out_p"].astype(np.float32)
        out[b] = acc
    if _trace:
        return out, res
    return out
